# revision 50
# baseline (speedup 1.0000x reference)
"""Conv3d (8,32,48,48,48) * (64,32,3,3,3) -> (8,64,46,46,46), valid, stride 1.

Strategy: data-parallel over batch (1 image per NeuronCore, 8 cores).
Per core the conv is an implicit GEMM:
  out[co, d, h', w'] = sum_{kd,ci,kh,kw} W[co,ci,kd,kh,kw] * X[ci, d+kd, h'+kh, w'+kw]
- a SBUF "window" tile holds FOUR input planes d..d+3 stacked on the full
  128 partitions (plane-major, (g, ci) with g = plane index in window),
- one K=128 matmul per (kh,kw) tap computes BOTH output planes d (cols
  0-63, weights zero on partition group 3) and d+1 (cols 64-127, weights
  zero on group 0) from a single shared rhs stream; kh, kw are free-dim
  offsets into the window tile (rows step 48). Same PE streaming cycles
  as two col-tiled K=96 matmuls, but half the instructions, one DMA and
  one DMA-completion semaphore per plane pair (so no wait-absorber dummy
  matmuls are needed: first matmul of a window carries the window sem on
  its LDWEIGHTS via bacc's move_matmul_waits_to_ldweights, psum sem on
  the MATMUL), and NumWeights==128 keeps fast-weight-load enabled,
- 9 accumulating matmuls per PSUM bank chunk ([128, rows*46] f32),
- drains: full-width [128, N] f32 PSUM + bias -> fp16 SBUF, alternating
  ScalarE activation / VectorE tensor_scalar per chunk so each PSUM
  buffer has exactly ONE reader (one release sem),
- PE warm-up: ~3.6us of N=128 matmuls on a zeroed tile, issued before
  the first input DMA lands, so the HAM clock gate (which needs ~3.4us
  of sustained PE activity) opens during warm-up and the real stream
  never pays the cold 1.2 GHz window,
- startup loads split across both HWDGE queues (weights+bias on sync,
  window 0 in four h-slices then windows 1-2 on scalar) so transfers
  don't contend and chunk 0 can start ~2.8us after the queues open,
- output staged as fp16 [2*CO, 23*SPP] (row = (parity, co)); one DMA per
  plane pair, except the last pair which drains in finer chunks and
  stores in three pieces so the final transfer (which gates the fixed
  ~9us teardown: all-engine barrier + per-sem clear storm) is short;
  host reinterleaves parities and upcasts to f32.
"""

import functools
import os

import numpy as np

import concourse.bacc as bacc
import concourse.tile as tile
from concourse import mybir
from concourse.ap import AP
from concourse.bass_utils import run_bass_kernel_spmd

# Problem constants (hardcoded per harness contract)
B = 8
CI = 32
DIN = 48
CO = 64
K = 3
DOUT = DIN - K + 1  # 46
SPP = DOUT * DOUT  # 2116 spatial positions per output plane
PLANE = DIN * DIN  # 2304 elements per (ci, plane)
NPAIR = DOUT // 2  # 23 plane pairs
NW = 4 * CI  # 128 window partitions (4 planes)

# h'-row chunking of a 46x46 output plane into PSUM-bank-sized matmuls
CHUNKS = [(0, 10), (10, 9), (19, 9), (28, 9), (37, 9)]  # (h0, rows) -> N = rows*46
# last pair: finer trailing chunks so the final drain+store tail is shorter
# ((37,5),(42,4) measured best: smaller final chunks hit the ~0.8us
# per-DMA descriptor floor with narrower bursts and lengthen the tail)
CHUNKS_TAIL = [(0, 10), (10, 9), (19, 9), (28, 9), (37, 5), (42, 4)]


F32 = mybir.dt.float32
F16 = mybir.dt.float16
BF16 = mybir.dt.bfloat16

MODE = os.environ.get("CONV_MM_DT", "fp16")
MM_DT = BF16 if MODE == "bf16" else F16

# >= ~3.5us of contiguous warm-up matmuls: the HAM clock gate needs ~3.4us
# of sustained PE activity to open (4/8 -> 8/8), and it must open BEFORE any
# idle gap (waiting for the first input DMA) or the first ~13 real taps run
# at 1.2 GHz. N=460 warm-ups hide their LDWEIGHTS completely (~96% PE-busy
# vs ~85% for N=128), so the gate's busy-time accumulates at full rate and
# the flip lands during warm-up, not in the real stream.
N_WARMUP = int(os.environ.get("CONV_WARMUP", "9"))


def _pack_mm(a):
    """Host-side cast of a float32 array to the matmul operand format."""
    if MODE == "bf16":
        import ml_dtypes

        return np.ascontiguousarray(a.astype(ml_dtypes.bfloat16))
    return np.ascontiguousarray(a.astype(np.float16))


@functools.lru_cache(maxsize=1)
def build_program():
    nc = bacc.Bacc("TRN2", target_bir_lowering=False, debug=False)

    x = nc.dram_tensor("x", [DIN * CI, PLANE], MM_DT, kind="ExternalInput").ap()
    # weights: row (g,ci), col (tap, parity, co); parity 0 uses groups 0-2
    # (kd=g), parity 1 uses groups 1-3 (kd=g-1); other group rows are zero.
    wt = nc.dram_tensor("wt", [NW, 9 * 2 * CO], MM_DT, kind="ExternalInput").ap()
    b2 = nc.dram_tensor("b2", [2 * CO, 1], F32, kind="ExternalInput").ap()
    # y row (j*CO+co) = output plane 2t+j, channel co, position t*SPP+pos
    y = nc.dram_tensor("y", [2 * CO, NPAIR * SPP], F16, kind="ExternalOutput").ap()

    with tile.TileContext(nc) as tc:
        with (
            tc.tile_pool(name="wp", bufs=1) as wpool,
            tc.tile_pool(name="xp", bufs=3) as xpool,
            tc.tile_pool(name="op", bufs=3) as opool,
            tc.tile_pool(name="pa", bufs=7, space="PSUM") as papool,
            tc.tile_pool(name="ps", bufs=1, space="PSUM") as pspool,
        ):
            # Never-read scratch PSUM bank for warm-up matmuls.
            scr = pspool.tile([2 * CO, 512], F32)

            # PE warm-up: N=460 matmuls on the framework's bf16 const tile
            # (memset + barriered inside Bass.__init__), read through
            # 0-stride broadcast APs so the warm-ups carry NO semaphore
            # waits at all and dispatch the moment the PE queue preamble
            # ends -- ~2us before the first input DMA lands. They keep the
            # PE array fully busy so the HAM clock gate opens (4/8 -> 8/8)
            # before the real stream starts.
            cb = nc.const_aps.aps[(BF16, 1.0)]
            cb_lhs = AP(cb.tensor, cb.offset, [[1, NW], [0, NW]])
            cb_rhs = AP(cb.tensor, cb.offset, [[1, NW], [0, 460]])
            for _ in range(N_WARMUP):
                nc.tensor.matmul(
                    scr[:, :460], cb_lhs, cb_rhs, start=True, stop=True,
                    tile_position=(0, 0),
                )

            # Startup DMAs: weights + bias on the sync queue; window 0 in
            # three h-slices, then windows 1-2, serially on the scalar
            # queue (its own queue -> no engine contention with sync;
            # slice A covers all chunk-0 reads so the stream starts ~2.8us
            # after the queues open). Waits ride on LDWEIGHTS/MATMUL slots
            # (bacc moves extra matmul waits to the paired LDWEIGHTS), so
            # no absorber matmuls are needed.
            # weights split so taps 0-3 land ~0.7us before taps 4-8 (tap 4
            # of the first chunk isn't needed until ~0.8us into the stream;
            # its LDWEIGHTS carries the second weights sem). Split at 3 taps
            # measured a 0.5-0.9us tap-3 stall when the stream starts early.
            WS = 4 * 2 * CO
            wa = wpool.tile([NW, 9 * 2 * CO], MM_DT)
            nc.sync.dma_start(wa[:, :WS], wt[:, :WS])
            nc.sync.dma_start(wa[:, WS:], wt[:, WS:])
            bias_t = wpool.tile([2 * CO, 1], F32)
            nc.sync.dma_start(bias_t[:, :], b2)
            x0 = xpool.tile([NW, PLANE], MM_DT, tag="xw", name="x0")
            for sl in range(4):
                lo, hi = sl * 12 * DIN, (sl + 1) * 12 * DIN
                nc.scalar.dma_start(x0[:, lo:hi], x[0:NW, lo:hi])

            for t in range(NPAIR):
                d0 = 2 * t
                if t == 0:
                    xw = x0
                else:
                    xw = xpool.tile([NW, PLANE], MM_DT, tag="xw", name=f"x{t}")
                    eng = nc.scalar if t <= 2 else nc.sync
                    eng.dma_start(xw[:, :], x[CI * d0 : CI * d0 + NW, :])
                x3 = xw[:, :].rearrange("p (h w) -> p h w", w=DIN)

                ot = opool.tile([2 * CO, SPP], F16)

                chunks = CHUNKS_TAIL if t == NPAIR - 1 else CHUNKS
                for ci_, (h0, rows) in enumerate(chunks):
                    n = rows * DOUT
                    pt = papool.tile([2 * CO, 512], F32, tag="pa")
                    pc = pt[:, :n]
                    for kh in range(K):
                        for kw in range(K):
                            first = kh == 0 and kw == 0
                            last = kh == K - 1 and kw == K - 1
                            tap = kh * K + kw
                            lhs = wa[:, tap * 2 * CO : (tap + 1) * 2 * CO]
                            rhs = x3[:, h0 + kh : h0 + kh + rows, kw : kw + DOUT]
                            nc.tensor.matmul(
                                pc[:, :], lhs, rhs, start=first, stop=last,
                                tile_position=(0, 0),
                            )
                    cs = slice(h0 * DOUT, h0 * DOUT + n)
                    # full-width drain, alternating engines per chunk so each
                    # PSUM buffer has a single reader
                    if ci_ % 2 == 0:
                        nc.scalar.activation(
                            ot[:, cs], pc[:, :],
                            mybir.ActivationFunctionType.Identity,
                            bias=bias_t[:, :],
                        )
                    else:
                        nc.vector.tensor_scalar_add(ot[:, cs], pc[:, :], bias_t[:, :])
                if t < NPAIR - 1:
                    # one store per plane pair
                    nc.sync.dma_start(y[:, t * SPP : (t + 1) * SPP], ot[:, :])
                else:
                    # last pair: store in three pieces, each issued as soon
                    # as its chunks have drained, so the final transfer
                    # (which gates the teardown barrier) is short
                    yt = y[:, t * SPP : (t + 1) * SPP]
                    HS1 = 28 * DOUT  # chunks 0-2
                    HS2 = 42 * DOUT  # chunks 3-4
                    nc.sync.dma_start(yt[:, :HS1], ot[:, :HS1])
                    nc.sync.dma_start(yt[:, HS1:HS2], ot[:, HS1:HS2])
                    # last piece on the scalar queue: its dispatch overlaps
                    # the final (vector) drain instead of queueing behind
                    # the HS1/HS2 stores on sync
                    nc.scalar.dma_start(yt[:, HS2:], ot[:, HS2:])

    nc.compile()
    return nc


def make_in_maps(inputs, weight, bias):
    """Host-side shard/pack: returns per-core input maps."""
    inputs = np.ascontiguousarray(np.asarray(inputs, dtype=np.float32))
    weight = np.asarray(weight, dtype=np.float32)
    bias = np.asarray(bias, dtype=np.float32)
    # weights: [(g,ci), (kh,kw,parity,co)]; parity 0 (even plane) takes
    # kd = g for g in 0..2, parity 1 (odd plane) takes kd = g-1 for g in
    # 1..3; remaining rows stay zero.
    wt = np.zeros((4, CI, 9, 2, CO), dtype=np.float32)
    wk = weight.transpose(2, 1, 3, 4, 0).reshape(K, CI, 9, CO)  # [kd,ci,tap,co]
    wt[0:3, :, :, 0, :] = wk
    wt[1:4, :, :, 1, :] = wk
    wt = _pack_mm(wt.reshape(NW, 9 * 2 * CO))
    b2 = np.ascontiguousarray(np.tile(bias, 2).reshape(2 * CO, 1))
    in_maps = []
    for c in range(B):
        xc = _pack_mm(inputs[c].transpose(1, 0, 2, 3).reshape(DIN * CI, PLANE))
        in_maps.append({"x": xc, "wt": wt, "b2": b2})
    return in_maps


def kernel(inputs, weight, bias, **run_kwargs):
    nc = build_program()
    in_maps = make_in_maps(inputs, weight, bias)
    res = run_bass_kernel_spmd(nc, in_maps, core_ids=list(range(B)), **run_kwargs)
    out = np.empty((B, CO, DOUT, DOUT, DOUT), dtype=np.float32)
    for c in range(B):
        yv = res.results[c]["y"].astype(np.float32).reshape(2, CO, NPAIR, DOUT, DOUT)
        out[c, :, 0::2] = yv[0]
        out[c, :, 1::2] = yv[1]
    return out


# revision 52
# speedup vs baseline: 1.0015x; 1.0015x over previous
"""Conv3d (8,32,48,48,48) * (64,32,3,3,3) -> (8,64,46,46,46), valid, stride 1.

Strategy: data-parallel over batch (1 image per NeuronCore, 8 cores).
Per core the conv is an implicit GEMM:
  out[co, d, h', w'] = sum_{kd,ci,kh,kw} W[co,ci,kd,kh,kw] * X[ci, d+kd, h'+kh, w'+kw]
- a SBUF "window" tile holds FOUR input planes d..d+3 stacked on the full
  128 partitions (plane-major, (g, ci) with g = plane index in window),
- one K=128 matmul per (kh,kw) tap computes BOTH output planes d (cols
  0-63, weights zero on partition group 3) and d+1 (cols 64-127, weights
  zero on group 0) from a single shared rhs stream; kh, kw are free-dim
  offsets into the window tile (rows step 48). Same PE streaming cycles
  as two col-tiled K=96 matmuls, but half the instructions, one DMA and
  one DMA-completion semaphore per plane pair (so no wait-absorber dummy
  matmuls are needed: first matmul of a window carries the window sem on
  its LDWEIGHTS via bacc's move_matmul_waits_to_ldweights, psum sem on
  the MATMUL), and NumWeights==128 keeps fast-weight-load enabled,
- 9 accumulating matmuls per PSUM bank chunk ([128, rows*46] f32),
- drains: full-width [128, N] f32 PSUM + bias -> fp16 SBUF, alternating
  ScalarE activation / VectorE tensor_scalar per chunk so each PSUM
  buffer has exactly ONE reader (one release sem),
- PE warm-up: ~3.6us of N=128 matmuls on a zeroed tile, issued before
  the first input DMA lands, so the HAM clock gate (which needs ~3.4us
  of sustained PE activity) opens during warm-up and the real stream
  never pays the cold 1.2 GHz window,
- startup loads split across both HWDGE queues (weights+bias on sync,
  window 0 in four h-slices then windows 1-2 on scalar) so transfers
  don't contend and chunk 0 can start ~2.8us after the queues open,
- output staged as fp16 [2*CO, 23*SPP] (row = (parity, co)); one DMA per
  plane pair, except the last pair which drains in finer chunks and
  stores in three pieces so the final transfer (which gates the fixed
  ~9us teardown: all-engine barrier + per-sem clear storm) is short;
  host reinterleaves parities and upcasts to f32.
"""

import functools
import os

import numpy as np

import concourse.bacc as bacc
import concourse.tile as tile
from concourse import mybir
from concourse.ap import AP
from concourse.bass_utils import run_bass_kernel_spmd

# Problem constants (hardcoded per harness contract)
B = 8
CI = 32
DIN = 48
CO = 64
K = 3
DOUT = DIN - K + 1  # 46
SPP = DOUT * DOUT  # 2116 spatial positions per output plane
PLANE = DIN * DIN  # 2304 elements per (ci, plane)
NPAIR = DOUT // 2  # 23 plane pairs
NW = 4 * CI  # 128 window partitions (4 planes)

# h'-row chunking of a 46x46 output plane into PSUM-bank-sized matmuls
CHUNKS = [(0, 10), (10, 9), (19, 9), (28, 9), (37, 9)]  # (h0, rows) -> N = rows*46
# last pair: finer trailing chunks so the final drain+store tail is shorter
# ((37,5),(42,4) measured best: smaller final chunks hit the ~0.8us
# per-DMA descriptor floor with narrower bursts and lengthen the tail)
CHUNKS_TAIL = [(0, 10), (10, 9), (19, 9), (28, 9), (37, 5), (42, 4)]


F32 = mybir.dt.float32
F16 = mybir.dt.float16
BF16 = mybir.dt.bfloat16

MODE = os.environ.get("CONV_MM_DT", "fp16")
MM_DT = BF16 if MODE == "bf16" else F16

# >= ~3.5us of contiguous warm-up matmuls: the HAM clock gate needs ~3.4us
# of sustained PE activity to open (4/8 -> 8/8), and it must open BEFORE any
# idle gap (waiting for the first input DMA) or the first ~13 real taps run
# at 1.2 GHz. N=460 warm-ups hide their LDWEIGHTS completely (~96% PE-busy
# vs ~85% for N=128), so the gate's busy-time accumulates at full rate and
# the flip lands during warm-up, not in the real stream.
N_WARMUP = int(os.environ.get("CONV_WARMUP", "9"))


def _pack_mm(a):
    """Host-side cast of a float32 array to the matmul operand format."""
    if MODE == "bf16":
        import ml_dtypes

        return np.ascontiguousarray(a.astype(ml_dtypes.bfloat16))
    return np.ascontiguousarray(a.astype(np.float16))


@functools.lru_cache(maxsize=1)
def build_program():
    nc = bacc.Bacc("TRN2", target_bir_lowering=False, debug=False)

    x = nc.dram_tensor("x", [DIN * CI, PLANE], MM_DT, kind="ExternalInput").ap()
    # weights: row (g,ci), col (tap, parity, co); parity 0 uses groups 0-2
    # (kd=g), parity 1 uses groups 1-3 (kd=g-1); other group rows are zero.
    wt = nc.dram_tensor("wt", [NW, 9 * 2 * CO], MM_DT, kind="ExternalInput").ap()
    b2 = nc.dram_tensor("b2", [2 * CO, 1], F32, kind="ExternalInput").ap()
    # y row (j*CO+co) = output plane 2t+j, channel co, position t*SPP+pos
    y = nc.dram_tensor("y", [2 * CO, NPAIR * SPP], F16, kind="ExternalOutput").ap()

    with tile.TileContext(nc) as tc:
        with (
            tc.tile_pool(name="wp", bufs=1) as wpool,
            tc.tile_pool(name="xp", bufs=3) as xpool,
            tc.tile_pool(name="op", bufs=3) as opool,
            tc.tile_pool(name="pa", bufs=7, space="PSUM") as papool,
            tc.tile_pool(name="ps", bufs=1, space="PSUM") as pspool,
        ):
            # Never-read scratch PSUM bank for warm-up matmuls.
            scr = pspool.tile([2 * CO, 512], F32)

            # PE warm-up: N=460 matmuls on the framework's bf16 const tile
            # (memset + barriered inside Bass.__init__), read through
            # 0-stride broadcast APs so the warm-ups carry NO semaphore
            # waits at all and dispatch the moment the PE queue preamble
            # ends -- ~2us before the first input DMA lands. They keep the
            # PE array fully busy so the HAM clock gate opens (4/8 -> 8/8)
            # before the real stream starts.
            cb = nc.const_aps.aps[(BF16, 1.0)]
            cb_lhs = AP(cb.tensor, cb.offset, [[1, NW], [0, NW]])
            cb_rhs = AP(cb.tensor, cb.offset, [[1, NW], [0, 460]])
            for _ in range(N_WARMUP):
                nc.tensor.matmul(
                    scr[:, :460], cb_lhs, cb_rhs, start=True, stop=True,
                    tile_position=(0, 0),
                )

            # Startup DMAs: weights + bias on the sync queue; window 0 in
            # three h-slices, then windows 1-2, serially on the scalar
            # queue (its own queue -> no engine contention with sync;
            # slice A covers all chunk-0 reads so the stream starts ~2.8us
            # after the queues open). Waits ride on LDWEIGHTS/MATMUL slots
            # (bacc moves extra matmul waits to the paired LDWEIGHTS), so
            # no absorber matmuls are needed.
            # weights split at 4 taps: taps 0-3 (sync, first) gate the
            # stream start together with window slice A; taps 4-8 go on the
            # SCALAR queue right behind slice A so they land ~+5.4us, just
            # before tap 4 of chunk 0 needs them at ~+5.55us (serialized
            # behind wa1 on sync they landed ~+6.5us and stalled the stream
            # ~1us every run -- measured: tap-4 LDWEIGHTS evt_wait 1794ns).
            # Tap-4's LDWEIGHTS carries the second weights sem.
            WS = 4 * 2 * CO
            wa = wpool.tile([NW, 9 * 2 * CO], MM_DT)
            nc.sync.dma_start(wa[:, :WS], wt[:, :WS])
            bias_t = wpool.tile([2 * CO, 1], F32)
            nc.sync.dma_start(bias_t[:, :], b2)
            x0 = xpool.tile([NW, PLANE], MM_DT, tag="xw", name="x0")
            nc.scalar.dma_start(x0[:, : 12 * DIN], x[0:NW, : 12 * DIN])
            nc.scalar.dma_start(wa[:, WS:], wt[:, WS:])
            for sl in range(1, 4):
                lo, hi = sl * 12 * DIN, (sl + 1) * 12 * DIN
                nc.scalar.dma_start(x0[:, lo:hi], x[0:NW, lo:hi])

            for t in range(NPAIR):
                d0 = 2 * t
                if t == 0:
                    xw = x0
                else:
                    xw = xpool.tile([NW, PLANE], MM_DT, tag="xw", name=f"x{t}")
                    eng = nc.scalar if t <= 2 else nc.sync
                    eng.dma_start(xw[:, :], x[CI * d0 : CI * d0 + NW, :])
                x3 = xw[:, :].rearrange("p (h w) -> p h w", w=DIN)

                ot = opool.tile([2 * CO, SPP], F16)

                chunks = CHUNKS_TAIL if t == NPAIR - 1 else CHUNKS
                for ci_, (h0, rows) in enumerate(chunks):
                    n = rows * DOUT
                    pt = papool.tile([2 * CO, 512], F32, tag="pa")
                    pc = pt[:, :n]
                    for kh in range(K):
                        for kw in range(K):
                            first = kh == 0 and kw == 0
                            last = kh == K - 1 and kw == K - 1
                            tap = kh * K + kw
                            lhs = wa[:, tap * 2 * CO : (tap + 1) * 2 * CO]
                            rhs = x3[:, h0 + kh : h0 + kh + rows, kw : kw + DOUT]
                            nc.tensor.matmul(
                                pc[:, :], lhs, rhs, start=first, stop=last,
                                tile_position=(0, 0),
                            )
                    cs = slice(h0 * DOUT, h0 * DOUT + n)
                    # full-width drain, alternating engines per chunk so each
                    # PSUM buffer has a single reader
                    if ci_ % 2 == 0:
                        nc.scalar.activation(
                            ot[:, cs], pc[:, :],
                            mybir.ActivationFunctionType.Identity,
                            bias=bias_t[:, :],
                        )
                    else:
                        nc.vector.tensor_scalar_add(ot[:, cs], pc[:, :], bias_t[:, :])
                if t < NPAIR - 1:
                    # one store per plane pair
                    nc.sync.dma_start(y[:, t * SPP : (t + 1) * SPP], ot[:, :])
                else:
                    # last pair: store in three pieces, each issued as soon
                    # as its chunks have drained, so the final transfer
                    # (which gates the teardown barrier) is short
                    yt = y[:, t * SPP : (t + 1) * SPP]
                    HS1 = 28 * DOUT  # chunks 0-2
                    HS2 = 42 * DOUT  # chunks 3-4
                    nc.sync.dma_start(yt[:, :HS1], ot[:, :HS1])
                    nc.sync.dma_start(yt[:, HS1:HS2], ot[:, HS1:HS2])
                    # last piece on the scalar queue: its dispatch overlaps
                    # the final (vector) drain instead of queueing behind
                    # the HS1/HS2 stores on sync
                    nc.scalar.dma_start(yt[:, HS2:], ot[:, HS2:])

    nc.compile()
    return nc


def make_in_maps(inputs, weight, bias):
    """Host-side shard/pack: returns per-core input maps."""
    inputs = np.ascontiguousarray(np.asarray(inputs, dtype=np.float32))
    weight = np.asarray(weight, dtype=np.float32)
    bias = np.asarray(bias, dtype=np.float32)
    # weights: [(g,ci), (kh,kw,parity,co)]; parity 0 (even plane) takes
    # kd = g for g in 0..2, parity 1 (odd plane) takes kd = g-1 for g in
    # 1..3; remaining rows stay zero.
    wt = np.zeros((4, CI, 9, 2, CO), dtype=np.float32)
    wk = weight.transpose(2, 1, 3, 4, 0).reshape(K, CI, 9, CO)  # [kd,ci,tap,co]
    wt[0:3, :, :, 0, :] = wk
    wt[1:4, :, :, 1, :] = wk
    wt = _pack_mm(wt.reshape(NW, 9 * 2 * CO))
    b2 = np.ascontiguousarray(np.tile(bias, 2).reshape(2 * CO, 1))
    in_maps = []
    for c in range(B):
        xc = _pack_mm(inputs[c].transpose(1, 0, 2, 3).reshape(DIN * CI, PLANE))
        in_maps.append({"x": xc, "wt": wt, "b2": b2})
    return in_maps


def kernel(inputs, weight, bias, **run_kwargs):
    nc = build_program()
    in_maps = make_in_maps(inputs, weight, bias)
    res = run_bass_kernel_spmd(nc, in_maps, core_ids=list(range(B)), **run_kwargs)
    out = np.empty((B, CO, DOUT, DOUT, DOUT), dtype=np.float32)
    for c in range(B):
        yv = res.results[c]["y"].astype(np.float32).reshape(2, CO, NPAIR, DOUT, DOUT)
        out[c, :, 0::2] = yv[0]
        out[c, :, 1::2] = yv[1]
    return out


# revision 53
# speedup vs baseline: 1.0037x; 1.0022x over previous
"""Conv3d (8,32,48,48,48) * (64,32,3,3,3) -> (8,64,46,46,46), valid, stride 1.

Strategy: data-parallel over batch (1 image per NeuronCore, 8 cores).
Per core the conv is an implicit GEMM:
  out[co, d, h', w'] = sum_{kd,ci,kh,kw} W[co,ci,kd,kh,kw] * X[ci, d+kd, h'+kh, w'+kw]
- a SBUF "window" tile holds FOUR input planes d..d+3 stacked on the full
  128 partitions (plane-major, (g, ci) with g = plane index in window),
- one K=128 matmul per (kh,kw) tap computes BOTH output planes d (cols
  0-63, weights zero on partition group 3) and d+1 (cols 64-127, weights
  zero on group 0) from a single shared rhs stream; kh, kw are free-dim
  offsets into the window tile (rows step 48). Same PE streaming cycles
  as two col-tiled K=96 matmuls, but half the instructions, one DMA and
  one DMA-completion semaphore per plane pair (so no wait-absorber dummy
  matmuls are needed: first matmul of a window carries the window sem on
  its LDWEIGHTS via bacc's move_matmul_waits_to_ldweights, psum sem on
  the MATMUL), and NumWeights==128 keeps fast-weight-load enabled,
- 9 accumulating matmuls per PSUM bank chunk ([128, rows*46] f32),
- drains: full-width [128, N] f32 PSUM + bias -> fp16 SBUF, alternating
  ScalarE activation / VectorE tensor_scalar per chunk so each PSUM
  buffer has exactly ONE reader (one release sem),
- PE warm-up: ~3.6us of N=128 matmuls on a zeroed tile, issued before
  the first input DMA lands, so the HAM clock gate (which needs ~3.4us
  of sustained PE activity) opens during warm-up and the real stream
  never pays the cold 1.2 GHz window,
- startup loads split across both HWDGE queues (weights+bias on sync,
  window 0 in four h-slices then windows 1-2 on scalar) so transfers
  don't contend and chunk 0 can start ~2.8us after the queues open,
- output staged as fp16 [2*CO, 23*SPP] (row = (parity, co)); one DMA per
  plane pair, except the last pair which drains in finer chunks and
  stores in three pieces so the final transfer (which gates the fixed
  ~9us teardown: all-engine barrier + per-sem clear storm) is short;
  host reinterleaves parities and upcasts to f32.
"""

import functools
import os

import numpy as np

import concourse.bacc as bacc
import concourse.tile as tile
from concourse import mybir
from concourse.ap import AP
from concourse.bass_utils import run_bass_kernel_spmd

# Problem constants (hardcoded per harness contract)
B = 8
CI = 32
DIN = 48
CO = 64
K = 3
DOUT = DIN - K + 1  # 46
SPP = DOUT * DOUT  # 2116 spatial positions per output plane
PLANE = DIN * DIN  # 2304 elements per (ci, plane)
NPAIR = DOUT // 2  # 23 plane pairs
NW = 4 * CI  # 128 window partitions (4 planes)

# h'-row chunking of a 46x46 output plane into PSUM-bank-sized matmuls
CHUNKS = [(0, 10), (10, 9), (19, 9), (28, 9), (37, 9)]  # (h0, rows) -> N = rows*46
# last pair: finer trailing chunks so the final drain+store tail is shorter
# ((37,5),(42,4) measured best: smaller final chunks hit the ~0.8us
# per-DMA descriptor floor with narrower bursts and lengthen the tail)
CHUNKS_TAIL = [(0, 10), (10, 9), (19, 9), (28, 9), (37, 5), (42, 4)]


F32 = mybir.dt.float32
F16 = mybir.dt.float16
BF16 = mybir.dt.bfloat16

MODE = os.environ.get("CONV_MM_DT", "fp16")
MM_DT = BF16 if MODE == "bf16" else F16

# >= ~3.5us of contiguous warm-up matmuls: the HAM clock gate needs ~3.4us
# of sustained PE activity to open (4/8 -> 8/8), and it must open BEFORE any
# idle gap (waiting for the first input DMA) or the first ~13 real taps run
# at 1.2 GHz. N=460 warm-ups hide their LDWEIGHTS completely (~96% PE-busy
# vs ~85% for N=128), so the gate's busy-time accumulates at full rate and
# the flip lands during warm-up, not in the real stream.
N_WARMUP = int(os.environ.get("CONV_WARMUP", "10"))


def _pack_mm(a):
    """Host-side cast of a float32 array to the matmul operand format."""
    if MODE == "bf16":
        import ml_dtypes

        return np.ascontiguousarray(a.astype(ml_dtypes.bfloat16))
    return np.ascontiguousarray(a.astype(np.float16))


@functools.lru_cache(maxsize=1)
def build_program():
    nc = bacc.Bacc("TRN2", target_bir_lowering=False, debug=False)

    x = nc.dram_tensor("x", [DIN * CI, PLANE], MM_DT, kind="ExternalInput").ap()
    # weights: row (g,ci), col (tap, parity, co); parity 0 uses groups 0-2
    # (kd=g), parity 1 uses groups 1-3 (kd=g-1); other group rows are zero.
    wt = nc.dram_tensor("wt", [NW, 9 * 2 * CO], MM_DT, kind="ExternalInput").ap()
    b2 = nc.dram_tensor("b2", [2 * CO, 1], F32, kind="ExternalInput").ap()
    # y row (j*CO+co) = output plane 2t+j, channel co, position t*SPP+pos
    y = nc.dram_tensor("y", [2 * CO, NPAIR * SPP], F16, kind="ExternalOutput").ap()

    with tile.TileContext(nc) as tc:
        with (
            tc.tile_pool(name="wp", bufs=1) as wpool,
            tc.tile_pool(name="xp", bufs=3) as xpool,
            tc.tile_pool(name="op", bufs=3) as opool,
            tc.tile_pool(name="pa", bufs=7, space="PSUM") as papool,
            tc.tile_pool(name="ps", bufs=1, space="PSUM") as pspool,
        ):
            # Never-read scratch PSUM bank for warm-up matmuls.
            scr = pspool.tile([2 * CO, 512], F32)

            # PE warm-up: N=460 matmuls on the framework's bf16 const tile
            # (memset + barriered inside Bass.__init__), read through
            # 0-stride broadcast APs so the warm-ups carry NO semaphore
            # waits at all and dispatch the moment the PE queue preamble
            # ends -- ~2us before the first input DMA lands. They keep the
            # PE array fully busy so the HAM clock gate opens (4/8 -> 8/8)
            # before the real stream starts.
            cb = nc.const_aps.aps[(BF16, 1.0)]
            cb_lhs = AP(cb.tensor, cb.offset, [[1, NW], [0, NW]])
            cb_rhs = AP(cb.tensor, cb.offset, [[1, NW], [0, 460]])
            for _ in range(N_WARMUP):
                nc.tensor.matmul(
                    scr[:, :460], cb_lhs, cb_rhs, start=True, stop=True,
                    tile_position=(0, 0),
                )

            # Startup DMAs: weights + bias on the sync queue; window 0 in
            # three h-slices, then windows 1-2, serially on the scalar
            # queue (its own queue -> no engine contention with sync;
            # slice A covers all chunk-0 reads so the stream starts ~2.8us
            # after the queues open). Waits ride on LDWEIGHTS/MATMUL slots
            # (bacc moves extra matmul waits to the paired LDWEIGHTS), so
            # no absorber matmuls are needed.
            # weights split at 4 taps: taps 0-3 (sync, first) gate the
            # stream start together with window slice A; taps 4-8 go on the
            # SCALAR queue right behind slice A so they land ~+5.4us, just
            # before tap 4 of chunk 0 needs them at ~+5.55us (serialized
            # behind wa1 on sync they landed ~+6.5us and stalled the stream
            # ~1us every run -- measured: tap-4 LDWEIGHTS evt_wait 1794ns).
            # Tap-4's LDWEIGHTS carries the second weights sem.
            WS = 4 * 2 * CO
            wa = wpool.tile([NW, 9 * 2 * CO], MM_DT)
            nc.sync.dma_start(wa[:, :WS], wt[:, :WS])
            bias_t = wpool.tile([2 * CO, 1], F32)
            nc.sync.dma_start(bias_t[:, :], b2)
            x0 = xpool.tile([NW, PLANE], MM_DT, tag="xw", name="x0")
            nc.scalar.dma_start(x0[:, : 12 * DIN], x[0:NW, : 12 * DIN])
            nc.scalar.dma_start(wa[:, WS:], wt[:, WS:])
            for sl in range(1, 4):
                lo, hi = sl * 12 * DIN, (sl + 1) * 12 * DIN
                nc.scalar.dma_start(x0[:, lo:hi], x[0:NW, lo:hi])

            for t in range(NPAIR):
                d0 = 2 * t
                if t == 0:
                    xw = x0
                else:
                    xw = xpool.tile([NW, PLANE], MM_DT, tag="xw", name=f"x{t}")
                    eng = nc.scalar if t <= 2 else nc.sync
                    eng.dma_start(xw[:, :], x[CI * d0 : CI * d0 + NW, :])
                x3 = xw[:, :].rearrange("p (h w) -> p h w", w=DIN)

                ot = opool.tile([2 * CO, SPP], F16)

                chunks = CHUNKS_TAIL if t == NPAIR - 1 else CHUNKS
                for ci_, (h0, rows) in enumerate(chunks):
                    n = rows * DOUT
                    pt = papool.tile([2 * CO, 512], F32, tag="pa")
                    pc = pt[:, :n]
                    for kh in range(K):
                        for kw in range(K):
                            first = kh == 0 and kw == 0
                            last = kh == K - 1 and kw == K - 1
                            tap = kh * K + kw
                            lhs = wa[:, tap * 2 * CO : (tap + 1) * 2 * CO]
                            rhs = x3[:, h0 + kh : h0 + kh + rows, kw : kw + DOUT]
                            nc.tensor.matmul(
                                pc[:, :], lhs, rhs, start=first, stop=last,
                                tile_position=(0, 0),
                            )
                    cs = slice(h0 * DOUT, h0 * DOUT + n)
                    # full-width drain, alternating engines per chunk so each
                    # PSUM buffer has a single reader
                    if ci_ % 2 == 0:
                        nc.scalar.activation(
                            ot[:, cs], pc[:, :],
                            mybir.ActivationFunctionType.Identity,
                            bias=bias_t[:, :],
                        )
                    else:
                        nc.vector.tensor_scalar_add(ot[:, cs], pc[:, :], bias_t[:, :])
                if t < NPAIR - 1:
                    # one store per plane pair
                    nc.sync.dma_start(y[:, t * SPP : (t + 1) * SPP], ot[:, :])
                else:
                    # last pair: store in three pieces, each issued as soon
                    # as its chunks have drained, so the final transfer
                    # (which gates the teardown barrier) is short
                    yt = y[:, t * SPP : (t + 1) * SPP]
                    HS1 = 28 * DOUT  # chunks 0-2
                    HS2 = 42 * DOUT  # chunks 3-4
                    nc.sync.dma_start(yt[:, :HS1], ot[:, :HS1])
                    nc.sync.dma_start(yt[:, HS1:HS2], ot[:, HS1:HS2])
                    # last piece on the scalar queue: its dispatch overlaps
                    # the final (vector) drain instead of queueing behind
                    # the HS1/HS2 stores on sync
                    nc.scalar.dma_start(yt[:, HS2:], ot[:, HS2:])

    nc.compile()
    return nc


def make_in_maps(inputs, weight, bias):
    """Host-side shard/pack: returns per-core input maps."""
    inputs = np.ascontiguousarray(np.asarray(inputs, dtype=np.float32))
    weight = np.asarray(weight, dtype=np.float32)
    bias = np.asarray(bias, dtype=np.float32)
    # weights: [(g,ci), (kh,kw,parity,co)]; parity 0 (even plane) takes
    # kd = g for g in 0..2, parity 1 (odd plane) takes kd = g-1 for g in
    # 1..3; remaining rows stay zero.
    wt = np.zeros((4, CI, 9, 2, CO), dtype=np.float32)
    wk = weight.transpose(2, 1, 3, 4, 0).reshape(K, CI, 9, CO)  # [kd,ci,tap,co]
    wt[0:3, :, :, 0, :] = wk
    wt[1:4, :, :, 1, :] = wk
    wt = _pack_mm(wt.reshape(NW, 9 * 2 * CO))
    b2 = np.ascontiguousarray(np.tile(bias, 2).reshape(2 * CO, 1))
    in_maps = []
    for c in range(B):
        xc = _pack_mm(inputs[c].transpose(1, 0, 2, 3).reshape(DIN * CI, PLANE))
        in_maps.append({"x": xc, "wt": wt, "b2": b2})
    return in_maps


def kernel(inputs, weight, bias, **run_kwargs):
    nc = build_program()
    in_maps = make_in_maps(inputs, weight, bias)
    res = run_bass_kernel_spmd(nc, in_maps, core_ids=list(range(B)), **run_kwargs)
    out = np.empty((B, CO, DOUT, DOUT, DOUT), dtype=np.float32)
    for c in range(B):
        yv = res.results[c]["y"].astype(np.float32).reshape(2, CO, NPAIR, DOUT, DOUT)
        out[c, :, 0::2] = yv[0]
        out[c, :, 1::2] = yv[1]
    return out
